# revision 9
# baseline (speedup 1.0000x reference)
"""CompGCN message-passing kernel for 8 Trainium2 NeuronCores.

Strategy (edge/1D graph partitioning by destination range):
  - Relabel nodes (load-balanced, serpentine by degree) into V = 8*GPC*128
    virtual ids; core j owns virtual nodes [j*GPC*128, (j+1)*GPC*128).
  - Pass B: edges bucketed by group-of-dst (128 nodes per group), streamed
    tile-by-tile (128 edges x [emb|1|v], bf16).  Per tile: one-hot matrix
    M[p, j] = (v_p == j) built on DVE (batched per quad), then PE matmul
    acc_group += E1^T @ M producing the transposed segment-sum [97, 128]
    (fp32 PSUM; row 96 = in-degree).  A host-pre-transposed copy of the
    stream ([97, 512] bf16 chunks, row 96 = ones) feeds
    he^T = [W_rel; b_rel]^T @ [E^T; 1]  -> he output in transposed bf16
    chunks (bias folded into the matmul).
  - Pass C: same scatter machinery keyed by src -> acc_src.
  - Final: per 128-node chunk (all fp32),
        h = indeg .* (h@W_O) + outdeg .* (h@W_I) - (esi@W_O + eso@W_I) + b
    PE for matmuls (h^T via PE transpose, degree columns via acc transposes),
    DVE scalar_tensor_tensor for the combines.
  - No collectives: each core owns its node range for both passes.

Host side: bucketing/padding of the two edge streams (+ transposed copy),
relabeling, and un-permutation of the outputs.
"""

import ml_dtypes
import numpy as np

import concourse.bacc as bacc
import concourse.bass as bass
import concourse.mybir as mybir
import concourse.tile as tile
from concourse.bass_utils import run_bass_kernel_spmd
from concourse.masks import make_identity

F32 = mybir.dt.float32
BF16 = mybir.dt.bfloat16
NPBF = ml_dtypes.bfloat16
TRACE = [False]
LAST_EXEC_NS = [None]
LAST_RES = [None]

P = 128  # partitions / edges per tile / nodes per group
REC = 98  # edge record: 96 emb + ones + v
DIM = 96
N_CORES = 8
QUAD = 4  # tiles per quad; also the DMA row-grouping factor
BLOCKS_PER_DMA = 8  # quads per edge-stream DMA (32 tiles)


# ----------------------------------------------------------------- host prep


def _make_rho(indeg, outdeg, n_cores, gpc):
    """Serpentine-balanced relabeling: virtual id rho[n] for each node."""
    n = indeg.shape[0]
    n_groups = n_cores * gpc
    v = n_groups * P
    deg = np.zeros(v, np.int64)
    deg[:n] = indeg + outdeg
    order = np.argsort(-deg, kind="stable")  # descending load
    snake = order.reshape(P, n_groups).copy()
    snake[1::2] = snake[1::2, ::-1]
    rho = np.empty(v, np.int64)
    rho[snake.T.ravel()] = np.arange(v)
    return rho


def _plan_pass(keys_v, n_cores, gpc):
    """Static tile schedule shared by all cores for one pass."""
    g_edge = keys_v // P
    pos = g_edge % gpc
    core = g_edge // gpc
    counts = np.zeros((n_cores, gpc), np.int64)
    np.add.at(counts, (core, pos), 1)
    tiles_k = np.ceil(counts.max(axis=0) / P).astype(np.int64)
    tiles_k = np.maximum(tiles_k, 1)
    t = int(tiles_k.sum())
    tiles_k[-1] += (-t) % QUAD
    return counts, tiles_k, int(tiles_k.sum())


def _fill_stream(edge_embs_bf, keys_v, n_cores, gpc, tiles_k, t_total):
    """Per-core padded streams [T*128, 98] bf16 plus edge->position maps."""
    g_edge = keys_v // P
    pos = g_edge % gpc
    core = g_edge // gpc
    offs = np.concatenate([[0], np.cumsum(tiles_k) * P])
    streams = np.zeros((n_cores, t_total * P, REC), NPBF)
    streams[:, :, 97] = -1.0  # pad marker
    order = np.lexsort((np.arange(keys_v.shape[0]), pos, core))
    core_s = core[order]
    pos_s = pos[order]
    bucket = core_s * gpc + pos_s
    uniq, start_idx = np.unique(bucket, return_index=True)
    rank = np.arange(order.shape[0])
    rank = rank - np.repeat(
        rank[start_idx], np.diff(np.concatenate([start_idx, [order.shape[0]]]))
    )
    padded_pos = offs[pos_s] + rank
    e_ids = order
    streams[core_s, padded_pos, :DIM] = edge_embs_bf[e_ids]
    streams[core_s, padded_pos, DIM] = 1.0
    streams[core_s, padded_pos, 97] = (keys_v[e_ids] % P).astype(NPBF)
    return streams, e_ids, core_s, padded_pos


def _group_stream(stream, t_total):
    """[T*128, REC] -> DMA-friendly grouped layout (QUAD rows per partition)."""
    return np.ascontiguousarray(
        stream.reshape(t_total // QUAD, QUAD, P, REC).swapaxes(1, 2)
    ).reshape(t_total * P, REC)


def _schedule(tiles_k):
    sched = []
    for k, nt in enumerate(tiles_k):
        for j in range(int(nt)):
            sched.append((k, j == 0, j == int(nt) - 1))
    return sched


# ------------------------------------------------------------- device build


def _emit_pass(nc, tc, pools, consts, edges_dram, sched, acc, etT_dram, heT_dram):
    edge_pool, m_pool, grp_ps, et_pool, he_ps, he_sb_pool = pools
    identity, iota4, w_relb_sb = consts
    do_he = heT_dram is not None
    t_total = len(sched)
    n_blocks = t_total // QUAD
    edges_v = edges_dram[:].rearrange("(b p g) r -> p b g r", p=P, g=QUAD)

    grp_tile = None
    alt = 0
    for b0 in range(0, n_blocks, BLOCKS_PER_DMA):
        nb = min(BLOCKS_PER_DMA, n_blocks - b0)
        ebuf = edge_pool.tile([P, BLOCKS_PER_DMA, QUAD, REC], BF16, tag="ebuf")
        nc.sync.dma_start(ebuf[:, :nb, :, :], edges_v[:, b0 : b0 + nb, :, :])
        for bi in range(nb):
            b = b0 + bi
            # batched one-hot build for the quad: [128, 4, 128]
            m4 = m_pool.tile([P, QUAD, P], BF16, tag="m4")
            nc.vector.tensor_tensor(
                m4[:],
                ebuf[:, bi, :, 97:98].to_broadcast([P, QUAD, P]),
                iota4[:],
                mybir.AluOpType.is_equal,
            )
            if do_he:
                etb = et_pool.tile([97, QUAD * P], BF16, tag="etb")
                nc.gpsimd.dma_start(etb[:], etT_dram[b])
                he_ps_tile = he_ps.tile([DIM, QUAD * P], F32, space="PSUM", tag="heps")
                nc.tensor.matmul(
                    he_ps_tile[:], w_relb_sb[:], etb[:], start=True, stop=True
                )
                he_sb = he_sb_pool.tile([DIM, QUAD * P], BF16, tag="hesb")
                if b % 2 == 0:
                    nc.scalar.copy(he_sb[:], he_ps_tile[:])
                else:
                    nc.vector.tensor_copy(he_sb[:], he_ps_tile[:])
                nc.gpsimd.dma_start(heT_dram[b], he_sb[:])
            for g in range(QUAD):
                t = b * QUAD + g
                k, first, last = sched[t]
                etile = ebuf[:, bi, g, :]
                if first:
                    grp_tile = grp_ps.tile([97, P], F32, space="PSUM", tag="grp")
                nc.tensor.matmul(
                    grp_tile[:], etile[:, 0:97], m4[:, g, :], start=first, stop=last
                )
                if last:
                    if alt == 0:
                        nc.scalar.copy(acc[:, k * P : (k + 1) * P], grp_tile[:])
                    else:
                        nc.vector.tensor_copy(acc[:, k * P : (k + 1) * P], grp_tile[:])
                    alt ^= 1


def build_program(gpc, t_b, t_c, sched_b, sched_c, num_devices=N_CORES):
    vc = gpc * P
    nq = t_b // QUAD
    nc = bacc.Bacc(
        "TRN2", target_bir_lowering=False, debug=False, num_devices=num_devices
    )

    edges_b = nc.dram_tensor("edges_b", [t_b * P, REC], BF16, kind="ExternalInput")
    edges_c = nc.dram_tensor("edges_c", [t_c * P, REC], BF16, kind="ExternalInput")
    etT_in = nc.dram_tensor("etT", [nq, 97, QUAD * P], BF16, kind="ExternalInput")
    nodes = nc.dram_tensor("nodes", [P, gpc * DIM], F32, kind="ExternalInput")
    iota_in = nc.dram_tensor("iota", [P, QUAD * P], BF16, kind="ExternalInput")
    w_relb_in = nc.dram_tensor("w_relb", [97, DIM], F32, kind="ExternalInput")
    w_o_in = nc.dram_tensor("w_o", [DIM, DIM], F32, kind="ExternalInput")
    wib_in = nc.dram_tensor("wib", [97, DIM], F32, kind="ExternalInput")

    h_out = nc.dram_tensor("h_out", [P, gpc * DIM], F32, kind="ExternalOutput")
    heT_out = nc.dram_tensor(
        "heT_out", [nq, DIM, QUAD * P], BF16, kind="ExternalOutput"
    )

    with tile.TileContext(nc) as tc:
        with (
            tc.tile_pool(name="const", bufs=1) as const_pool,
            tc.tile_pool(name="acc", bufs=1) as acc_pool,
        ):
            identity = const_pool.tile([P, P], F32)
            make_identity(nc, identity[:])
            iota4 = const_pool.tile([P, QUAD, P], BF16)
            nc.sync.dma_start(
                iota4[:], iota_in[:].rearrange("p (q i) -> p q i", q=QUAD)
            )
            w_relb_f32 = const_pool.tile([97, DIM], F32)
            nc.sync.dma_start(w_relb_f32[:], w_relb_in[:])
            w_relb_sb = const_pool.tile([97, DIM], BF16)
            nc.vector.tensor_copy(w_relb_sb[:], w_relb_f32[:])
            w_o_sb = const_pool.tile([DIM, DIM], F32)
            nc.sync.dma_start(w_o_sb[:], w_o_in[:])
            wib_sb = const_pool.tile([97, DIM], F32)
            nc.sync.dma_start(wib_sb[:], wib_in[:])

            acc_dst = acc_pool.tile([97, vc], F32)
            acc_src = acc_pool.tile([97, vc], F32)
            nbuf = acc_pool.tile([P, gpc * DIM], F32)
            nc.sync.dma_start(nbuf[:], nodes[:])
            hbuf = acc_pool.tile([P, gpc * DIM], F32)

            consts = (identity, iota4, w_relb_sb)
            with (
                tc.tile_pool(name="edge", bufs=3) as edge_pool,
                tc.tile_pool(name="m", bufs=4) as m_pool,
                tc.tile_pool(name="etb", bufs=3) as et_pool,
                tc.tile_pool(name="hesb", bufs=3) as he_sb_pool,
                tc.tile_pool(name="grp", bufs=3, space="PSUM") as grp_ps,
                tc.tile_pool(name="heps", bufs=2, space="PSUM") as he_ps,
            ):
                pools = (edge_pool, m_pool, grp_ps, et_pool, he_ps, he_sb_pool)
                _emit_pass(
                    nc, tc, pools, consts, edges_b, sched_b, acc_dst, etT_in, heT_out
                )
                _emit_pass(nc, tc, pools, consts, edges_c, sched_c, acc_src, None, None)

            # ---- final node phase (fp32) ------------------------------
            with (
                tc.tile_pool(name="fin", bufs=3) as fin_pool,
                tc.tile_pool(name="finps", bufs=2, space="PSUM") as fin_ps,
            ):
                for c in range(gpc):
                    cs = slice(c * P, (c + 1) * P)
                    dsl = slice(c * DIM, (c + 1) * DIM)
                    bank1 = fin_ps.tile([P, 512], F32, space="PSUM", tag="b1")
                    # hT [96,128] | acc_dst^T [128,97] | acc_src^T [128,97]
                    nc.tensor.transpose(bank1[0:DIM, 0:P], nbuf[:, dsl], identity[:])
                    nc.tensor.transpose(
                        bank1[:, 128:225], acc_dst[:, cs], identity[0:97, 0:97]
                    )
                    nc.tensor.transpose(
                        bank1[:, 256:353], acc_src[:, cs], identity[0:97, 0:97]
                    )
                    ht_sb = fin_pool.tile([DIM, P], F32, tag="htsb")
                    nc.scalar.copy(ht_sb[:], bank1[0:DIM, 0:P])

                    bank2 = fin_ps.tile([P, 512], F32, space="PSUM", tag="b2")
                    # p1a [:,0:96] | p1b [:,128:224] | p2 [:,256:352]
                    nc.tensor.matmul(
                        bank2[:, 0:DIM], ht_sb[:], w_o_sb[:], start=True, stop=True
                    )
                    nc.tensor.matmul(
                        bank2[:, 128 : 128 + DIM],
                        ht_sb[:],
                        wib_sb[0:DIM, :],
                        start=True,
                        stop=True,
                    )
                    e97 = fin_pool.tile([97, P], F32, tag="e97")
                    nc.vector.tensor_copy(e97[0:DIM, :], acc_src[0:DIM, cs])
                    nc.vector.memset(e97[DIM:97, :], 1.0)
                    nc.tensor.matmul(
                        bank2[:, 256 : 256 + DIM],
                        acc_dst[0:DIM, cs],
                        w_o_sb[:],
                        start=True,
                        stop=False,
                    )
                    nc.tensor.matmul(
                        bank2[:, 256 : 256 + DIM],
                        e97[:],
                        wib_sb[:],
                        start=False,
                        stop=True,
                    )

                    p1a_sb = fin_pool.tile([P, DIM], F32, tag="p1asb")
                    nc.scalar.copy(p1a_sb[:], bank2[:, 0:DIM])
                    p1b_sb = fin_pool.tile([P, DIM], F32, tag="p1bsb")
                    nc.scalar.copy(p1b_sb[:], bank2[:, 128 : 128 + DIM])
                    t1 = fin_pool.tile([P, DIM], F32, tag="t1")
                    nc.vector.scalar_tensor_tensor(
                        t1[:],
                        p1a_sb[:],
                        bank1[:, 128 + DIM : 128 + DIM + 1],
                        bank2[:, 256 : 256 + DIM],
                        mybir.AluOpType.mult,
                        mybir.AluOpType.subtract,
                    )
                    nc.vector.scalar_tensor_tensor(
                        hbuf[:, dsl],
                        p1b_sb[:],
                        bank1[:, 256 + DIM : 256 + DIM + 1],
                        t1[:],
                        mybir.AluOpType.mult,
                        mybir.AluOpType.add,
                    )
                nc.sync.dma_start(h_out[:], hbuf[:])

    nc.compile()
    return nc


# ------------------------------------------------------------------- driver


def kernel(node_embs, edge_embs, src, dst, W_O, b_O, W_I, b_I, W_rel, b_rel):
    node_embs = np.asarray(node_embs, np.float32)
    edge_embs = np.asarray(edge_embs, np.float32)
    src = np.asarray(src)
    dst = np.asarray(dst)
    n = node_embs.shape[0]
    n_edges = edge_embs.shape[0]
    gpc = int(np.ceil(n / (N_CORES * P)))
    vc = gpc * P
    v = vc * N_CORES

    indeg = np.bincount(dst, minlength=n)
    outdeg = np.bincount(src, minlength=n)
    rho = _make_rho(indeg, outdeg, N_CORES, gpc)

    dst_v = rho[dst]
    src_v = rho[src]
    _, tiles_b, t_b = _plan_pass(dst_v, N_CORES, gpc)
    _, tiles_c, t_c = _plan_pass(src_v, N_CORES, gpc)
    sched_b = _schedule(tiles_b)
    sched_c = _schedule(tiles_c)

    edge_bf = edge_embs.astype(NPBF)
    streams_b, eid_b, core_b, pos_b = _fill_stream(
        edge_bf, dst_v, N_CORES, gpc, tiles_b, t_b
    )
    streams_c, _, _, _ = _fill_stream(edge_bf, src_v, N_CORES, gpc, tiles_c, t_c)

    # pre-transposed he input: [nq, 97, 512], rows 0:96 = emb^T, row 96 = 1
    nq = t_b // QUAD
    etT = np.empty((N_CORES, nq, 97, QUAD * P), NPBF)
    for j in range(N_CORES):
        s = streams_b[j].reshape(nq, QUAD * P, REC)
        etT[j, :, 0:DIM, :] = s[:, :, 0:DIM].transpose(0, 2, 1)
        etT[j, :, DIM, :] = 1.0

    node_v = np.zeros((v, DIM), np.float32)
    node_v[rho[:n]] = node_embs

    iota4 = np.tile(np.arange(P, dtype=NPBF), (P, QUAD))
    w_relb = np.concatenate(
        [np.asarray(W_rel, np.float32), np.asarray(b_rel, np.float32)[None, :]], 0
    )
    wib = np.concatenate(
        [
            np.asarray(W_I, np.float32),
            -(np.asarray(b_O, np.float32) + np.asarray(b_I, np.float32))[None, :],
        ],
        0,
    )

    nc = build_program(gpc, t_b, t_c, sched_b, sched_c)

    in_maps = []
    for j in range(N_CORES):
        nodes_j = (
            node_v[j * vc : (j + 1) * vc]
            .reshape(gpc, P, DIM)
            .swapaxes(0, 1)
            .reshape(P, gpc * DIM)
        )
        in_maps.append(
            {
                "edges_b": _group_stream(streams_b[j], t_b),
                "edges_c": _group_stream(streams_c[j], t_c),
                "etT": etT[j],
                "nodes": np.ascontiguousarray(nodes_j),
                "iota": iota4,
                "w_relb": w_relb,
                "w_o": np.asarray(W_O, np.float32),
                "wib": wib,
            }
        )

    res = run_bass_kernel_spmd(
        nc, in_maps, core_ids=list(range(N_CORES)), trace=TRACE[0]
    )
    LAST_EXEC_NS[0] = res.exec_time_ns
    LAST_RES[0] = res

    h_parts = []
    for j in range(N_CORES):
        hj = res.results[j]["h_out"]  # [128, gpc*96]
        h_parts.append(hj.reshape(P, gpc, DIM).swapaxes(0, 1).reshape(vc, DIM))
    h_all = np.concatenate(h_parts, 0)
    h = h_all[rho[:n]].astype(np.float32)

    he = np.empty((n_edges, DIM), np.float32)
    for j in range(N_CORES):
        het = res.results[j]["heT_out"]  # [nq, 96, 512] bf16
        he_lin = het.astype(np.float32).transpose(0, 2, 1).reshape(-1, DIM)
        sel = core_b == j
        he[eid_b[sel]] = he_lin[pos_b[sel]]
    return h, he
